# revision 6
# baseline (speedup 1.0000x reference)
"""Trainium2 Bass kernel for nn_EquiConv2d (equirectangular deformable conv).

Key structural facts exploited (derived from the reference geometry):
  * off_y is exactly longitude-invariant, so each (tap k, row h) samples two
    fixed input rows (iy0, iy0+1) with a constant y-fraction.
  * off_x is longitude-invariant up to the 2*pi wrap: sampling along a row is
    a CIRCULAR shift by a constant s0(k,h) plus a constant x-fraction.
  * Hence the whole deformable conv is 18 matmuls per output row
    ([128=(c x row-pair) contraction, 512 free]) reading circularly
    duplicated row-pair tiles at per-(k,h) column offsets, with the bilinear
    corner weights folded into the stationary (weight) operand.
  * The per-(k,h) column offsets are per-core data: loaded into PE registers
    from an int32 table and applied as dynamic AP slices, so all 8 cores run
    ONE SPMD program.
  * Two fp32 oddities handled exactly: tap (k=7,h=255) is identically zero
    (py==256.0 -> all corners invalid) and tap (k=1,h=1) samples near the
    antipode with fp32-noise-scattered positions -> handled by 3 extra
    matmul slots with per-column coefficient vectors (data-driven, active
    only on the cores owning global row 1).

Sharding: 8 cores = 2 batches x 4 bands of 64 output rows.
"""

import math

import numpy as np

# ----------------------------------------------------------------------------
# problem constants
B, C, H, W = 2, 64, 256, 512
O, KH, KW = 64, 3, 3
K = KH * KW
NCORES = 8
NROW = 64            # output rows per core
NSLOT = 2 * K        # standard matmul slots per row
NSPEC = 3            # special (antipode) slots, accumulated into local row 1
RING = 16            # staged row-pair ring slots
PF = 3               # staging prefetch lead (rows)
SLOTW = 2048         # F(1024) + G(1024) columns per ring slot
GOFF = 1024

_CACHE = {}


# ----------------------------------------------------------------------------
# host-side geometry tables (must replicate reference fp32 semantics exactly)

def _compute_offsets_jax():
    """Bit-exact replica of reference.equi_offsets on jax CPU."""
    import jax
    import jax.numpy as jnp
    cpu = jax.devices("cpu")[0]
    with jax.default_device(cpu):
        dtype = jnp.float32
        pano_H, pano_W, kH, kW = H, W, KH, KW
        Kk = kH * kW
        u = jnp.arange(pano_W, dtype=dtype)
        v = jnp.arange(pano_H, dtype=dtype)
        phi = (u - pano_W / 2.0) / pano_W * (2.0 * math.pi)
        theta = -(v - pano_H / 2.0) / pano_H * math.pi
        cp, sp = jnp.cos(phi), jnp.sin(phi)
        z, one = jnp.zeros_like(cp), jnp.ones_like(cp)
        Ry = jnp.stack([jnp.stack([cp, z, sp], -1),
                        jnp.stack([z, one, z], -1),
                        jnp.stack([-sp, z, cp], -1)], -2)
        ct, st = jnp.cos(theta), jnp.sin(theta)
        zh, oh = jnp.zeros_like(ct), jnp.ones_like(ct)
        Rx = jnp.stack([jnp.stack([oh, zh, zh], -1),
                        jnp.stack([zh, ct, -st], -1),
                        jnp.stack([zh, st, ct], -1)], -2)
        ROT = jnp.einsum('wij,hjk->hwik', Ry, Rx)
        fov_w = kW * (2.0 * math.pi / pano_W)
        focal = (kW / 2.0) / math.tan(fov_w / 2.0)
        hg = (jnp.arange(kH, dtype=dtype)[:, None] + 0.5 - kH / 2.0)
        wg = (jnp.arange(kW, dtype=dtype)[None, :] + 0.5 - kW / 2.0)
        hg = jnp.broadcast_to(hg, (kH, kW)).reshape(Kk)
        wg = jnp.broadcast_to(wg, (kH, kW)).reshape(Kk)
        rays0 = jnp.stack([wg / focal, hg / focal, jnp.ones(Kk, dtype)], 0)
        rays0 = rays0 / jnp.linalg.norm(rays0, axis=0, keepdims=True)
        rays = jnp.einsum('hwik,kn->hwin', ROT, rays0)
        phi2 = jnp.arctan2(rays[..., 0, :], rays[..., 2, :])
        th2 = jnp.arcsin(jnp.clip(rays[..., 1, :], -1.0, 1.0))
        x = pano_W / (2.0 * math.pi) * phi2 + pano_W / 2.0
        y = pano_H / math.pi * th2 + pano_H / 2.0
        off_x = x - (wg[None, None, :] + u[None, :, None])
        off_y = y - (hg[None, None, :] + v[:, None, None])
        return (np.asarray(jnp.transpose(off_y, (2, 0, 1))),
                np.asarray(jnp.transpose(off_x, (2, 0, 1))))


def _build_tap_tables():
    off_y, off_x = _compute_offsets_jax()
    ky = np.repeat(np.arange(KH), KW).astype(np.float32)
    kx = np.tile(np.arange(KW), KH).astype(np.float32)
    base_x = (np.arange(W, dtype=np.float32) - np.float32(1))
    base_y = (np.arange(H, dtype=np.float32) - np.float32(1))
    px = (base_x[None, None, :] + kx[:, None, None] + off_x).astype(np.float32)
    py = (base_y[None, :, None] + ky[:, None, None] + off_y).astype(np.float32)
    pyc = py[:, :, 0]
    assert np.all(py == pyc[:, :, None]), "off_y not longitude-invariant"

    iy0 = np.floor(pyc).astype(np.int64)
    wy1 = (pyc - np.floor(pyc)).astype(np.float64)
    v0 = (iy0 >= 0) & (iy0 < H)
    v1 = (iy0 + 1 >= 0) & (iy0 + 1 < H)
    cy0 = np.where(v0, 1.0 - wy1, 0.0)
    cy1 = np.where(v1, wy1, 0.0)

    Draw = np.mod((px.astype(np.float64) - np.arange(W)[None, None, :]), 512.0)
    ang = Draw / 512.0 * 2 * np.pi
    mean = np.mod(np.angle(np.exp(1j * ang).mean(axis=2)) / (2 * np.pi) * 512.0,
                  512.0)
    resid = np.mod(Draw - mean[:, :, None] + 256.0, 512.0) - 256.0
    D = mean + np.median(resid, axis=2)
    s0 = np.mod(np.floor(D), 512).astype(np.int64)
    frac = D - np.floor(D)

    special = np.zeros((K, H), dtype=bool)
    special[1, 1] = True
    dead = (cy0 == 0.0) & (cy1 == 0.0)

    Ddev = np.abs(np.mod(Draw - D[:, :, None] + 256.0, 512.0) - 256.0)
    dev = Ddev.max(axis=2)
    bad = (dev > 5e-4) & ~special & ~dead
    assert not bad.any(), f"unrepresentable taps: {np.argwhere(bad)}"

    def ref_coefs(p):
        x0 = math.floor(p)
        fr = p - x0
        out = {}
        for ix, wt in ((x0, 1.0 - fr), (x0 + 1, fr)):
            if 0 <= ix < W and wt != 0.0:
                out[ix] = out.get(ix, 0.0) + wt
        return out

    # seam variant selection: decided by the exact fp32 px at the wrap column
    slot0_useG = np.zeros((K, H), dtype=bool)
    slot1_useF = np.zeros((K, H), dtype=bool)
    for k in range(K):
        for h in range(H):
            if special[k, h] or dead[k, h]:
                continue
            s = int(s0[k, h]); fr = frac[k, h]
            if s >= 1:
                w0 = (512 - s) % 512
                rc = ref_coefs(float(px[k, h, w0]))
                slot0_useG[k, h] = (abs(rc.get(0, 0.0))
                                    < abs(rc.get(0, 0.0) - (1 - fr)))
            w1 = (511 - s) % 512
            rc = ref_coefs(float(px[k, h, w1]))
            slot1_useF[k, h] = (abs(rc.get(0, 0.0) - fr)
                                < abs(rc.get(0, 0.0)))

    # special tap (1,1): per-column coefficients on F offsets 255..257
    pxs = px[1, 1, :].astype(np.float64)
    Gam = np.zeros((3, W), dtype=np.float64)
    for w in range(W):
        p = pxs[w]
        x0 = math.floor(p)
        fr = p - x0
        for ix, wt in ((x0, 1.0 - fr), (x0 + 1, fr)):
            if 0 <= ix < W and wt != 0.0:
                found = False
                for jj in range(3):
                    if (255 + jj + w) % 512 == ix % 512:
                        Gam[jj, w] += wt
                        found = True
                        break
                assert found, (w, p, ix)

    return dict(iy0=iy0, cy0=cy0, cy1=cy1, s0=s0, frac=frac,
                slot0_useG=slot0_useG, slot1_useF=slot1_useF,
                special=special, dead=dead, Gam=Gam)


# ----------------------------------------------------------------------------
# uniform SPMD schedule

def _build_schedule(tt):
    blocks = []
    for blk in range(4):
        h0 = blk * NROW
        ev_of, events, first_use = {}, [], []
        need = np.zeros((NROW, K), np.int64)
        for lh in range(NROW):
            for k in range(K):
                r = int(np.clip(tt['iy0'][k, h0 + lh], 0, 255))
                if r not in ev_of:
                    ev_of[r] = len(events)
                    events.append(r)
                    first_use.append(lh)
                need[lh, k] = ev_of[r]
        blocks.append(dict(events=events, first_use=first_use, need=need))

    E = max(len(b['events']) for b in blocks)
    for b in blocks:
        while len(b['events']) < E:
            b['events'].append(b['events'][-1])

    # uniform staged-count before row lh:  tgt(lh) = U[min(lh+PF, NROW-1)]
    U = np.zeros(NROW, np.int64)
    for lh in range(NROW):
        U[lh] = max(int(np.searchsorted(np.asarray(b['first_use']), lh, 'right'))
                    for b in blocks)
    tgt = np.array([U[min(lh + PF, NROW - 1)] for lh in range(NROW)])

    # ring-overwrite feasibility
    ls = np.full(E, NROW, np.int64)
    for e in range(E):
        hit = np.where(tgt > e)[0]
        if len(hit):
            ls[e] = hit[0]
    for b in blocks:
        lastuse = {}
        for lh in range(NROW):
            for k in range(K):
                lastuse[int(b['need'][lh, k])] = lh
        for e in range(RING, E):
            prev = e - RING
            if prev in lastuse:
                assert lastuse[prev] < ls[e], \
                    f"RING={RING} too small: ev{e} overwrites ev{prev} " \
                    f"(lastuse {lastuse[prev]}, staged before row {ls[e]})"
    espc = int(blocks[0]['need'][1, 1])
    return blocks, E, tgt, espc


def _build_core_tables(tt, blocks):
    """Per-block offset and scale tables."""
    offs, scs = [], []
    for blk in range(4):
        need = blocks[blk]['need']
        h0 = blk * NROW
        offt = np.zeros((NROW, NSLOT), np.int32)
        sc = np.zeros((128, NROW * NSLOT + NSPEC), np.float32)
        for lh in range(NROW):
            h = h0 + lh
            for k in range(K):
                base = int(need[lh, k] % RING) * SLOTW
                s = int(tt['s0'][k, h])
                if tt['dead'][k, h] or tt['special'][k, h]:
                    v0 = v1 = base
                else:
                    if tt['slot0_useG'][k, h] and s >= 1:
                        v0 = base + GOFF + s - 1
                    else:
                        v0 = base + s
                    v1 = base + s + 1 if tt['slot1_useF'][k, h] \
                        else base + GOFF + s
                    fr = tt['frac'][k, h]
                    c0, c1 = tt['cy0'][k, h], tt['cy1'][k, h]
                    col = lh * NSLOT + 2 * k
                    sc[:64, col] = np.float32(c0 * (1 - fr))
                    sc[64:, col] = np.float32(c1 * (1 - fr))
                    sc[:64, col + 1] = np.float32(c0 * fr)
                    sc[64:, col + 1] = np.float32(c1 * fr)
                offt[lh, 2 * k] = v0
                offt[lh, 2 * k + 1] = v1
        if blk == 0:
            sc[:64, NROW * NSLOT:NROW * NSLOT + NSPEC] = 1.0
        offs.append(offt.reshape(1, -1))
        scs.append(sc)
    return offs, scs


# ----------------------------------------------------------------------------
# device program

def _emit_section(tc, aps, tiles, tt, blkinfo, j):
    """Emit one per-band section (all-static APs)."""
    import concourse.mybir as mybir
    nc = tc.nc
    f32 = mybir.dt.float32
    buf, sct, w2t, coeft, biast = tiles
    xb, outd = aps['xb'], aps['out']
    need = blkinfo['need']
    first_use = blkinfo['first_use']
    E_j = len(first_use)

    cum = [int(np.searchsorted(np.asarray(first_use), lh, 'right'))
           for lh in range(NROW)]
    tgt = [cum[min(lh + PF, NROW - 1)] for lh in range(NROW)]

    # ring feasibility: event e staged before row ls(e) must not clobber a
    # slot still needed
    ls = [NROW] * E_j
    for e in range(E_j):
        for lh in range(NROW):
            if tgt[lh] > e:
                ls[e] = lh
                break
    lastuse = {}
    for lh in range(NROW):
        for k in range(K):
            lastuse[int(need[lh, k])] = lh
    for e in range(RING, E_j):
        if e - RING in lastuse:
            assert lastuse[e - RING] < ls[e], (j, e)

    def stage(e):
        base = (e % RING) * SLOTW
        src = xb[e].rearrange("p c w -> (p c) w")
        nc.sync.dma_start(buf[:, base:base + W], src)
        nc.sync.dma_start(buf[:, base + W:base + 2 * W], src)
        nc.vector.tensor_copy(buf[:, base + GOFF:base + GOFF + W],
                              buf[:, base + 1:base + 1 + W])
        nc.scalar.copy(buf[:, base + GOFF + W:base + GOFF + 2 * W],
                       buf[:, base + 1:base + 1 + W])
        nc.gpsimd.memset(buf[:, base + GOFF + 511:base + GOFF + 512], 0.0)

    psp, lhsp, zp, outp = tiles_pools[0]

    staged = 0
    for lh in range(NROW):
        while staged < tgt[lh]:
            stage(staged)
            staged += 1
        h = j * NROW + lh
        ps = psp.tile([O, W], f32, tag="ps")
        nmm = NSLOT + (NSPEC if (j == 0 and lh == 1) else 0)
        mi = 0
        for k in range(K):
            base = int(need[lh, k] % RING) * SLOTW
            s = int(tt['s0'][k, h])
            if tt['dead'][k, h] or tt['special'][k, h]:
                v0 = v1 = base
            else:
                if tt['slot0_useG'][k, h] and s >= 1:
                    v0 = base + GOFF + s - 1
                else:
                    v0 = base + s
                v1 = base + s + 1 if tt['slot1_useF'][k, h] \
                    else base + GOFF + s
            for sl, v in ((0, v0), (1, v1)):
                lt = lhsp.tile([128, O], f32, tag="lhst")
                col = lh * NSLOT + 2 * k + sl
                nc.vector.tensor_scalar_mul(
                    lt, w2t[:, k * O:(k + 1) * O], sct[:, col:col + 1])
                nc.tensor.matmul(ps, lt, buf[:, v:v + W],
                                 start=(mi == 0), stop=(mi == nmm - 1))
                mi += 1
        if j == 0 and lh == 1:
            sbase = int(need[1, 1] % RING) * SLOTW
            for jj in range(NSPEC):
                zt = zp.tile([128, W], f32, tag="spz")
                nc.vector.tensor_mul(
                    zt, buf[:, sbase + 255 + jj:sbase + 255 + jj + W],
                    coeft[:, jj * W:(jj + 1) * W])
                lt = lhsp.tile([128, O], f32, tag="lhst")
                colx = NROW * NSLOT + jj
                nc.vector.tensor_scalar_mul(
                    lt, w2t[:, 1 * O:2 * O], sct[:, colx:colx + 1])
                nc.tensor.matmul(ps, lt, zt, start=False,
                                 stop=(mi == nmm - 1))
                mi += 1
        ot = outp.tile([O, W], f32, tag="out")
        nc.scalar.activation(ot, ps,
                             mybir.ActivationFunctionType.Identity,
                             bias=biast, scale=1.0)
        nc.sync.dma_start(outd[lh], ot)


tiles_pools = [None]


def _emit_kernel(tc, aps, tt, blocks):
    import concourse.mybir as mybir
    nc = tc.nc
    f32 = mybir.dt.float32

    with tc.tile_pool(name="bigp", bufs=1) as bigp, \
         tc.tile_pool(name="lhsp", bufs=24) as lhsp, \
         tc.tile_pool(name="zp", bufs=3) as zp, \
         tc.tile_pool(name="psp", bufs=4, space="PSUM") as psp, \
         tc.tile_pool(name="outp", bufs=3) as outp:

        buf = bigp.tile([128, RING * SLOTW], f32)
        sct = bigp.tile([128, NROW * NSLOT + NSPEC], f32)
        w2t = bigp.tile([128, K * O], f32)
        coeft = bigp.tile([128, NSPEC * W], f32)
        biast = bigp.tile([O, 1], f32)

        nc.sync.dma_start(sct, aps['sc'])
        nc.sync.dma_start(w2t, aps['w2'])
        nc.sync.dma_start(coeft, aps['coefr'])
        nc.sync.dma_start(biast, aps['biasd'])

        blkv = nc.values_load(aps['blkid'][0:1, 0:1],
                              min_val=0, max_val=3,
                              skip_runtime_bounds_check=True)

        tiles = (buf, sct, w2t, coeft, biast)
        tiles_pools[0] = (psp, lhsp, zp, outp)
        for j in range(4):
            with tc.If(blkv == j):
                _emit_section(tc, aps, tiles, tt, blocks[j], j)


def _get_compiled():
    """Build tables, schedule, and the Bass program once."""
    if 'prog' in _CACHE:
        return _CACHE['prog']
    import concourse.mybir as mybir
    import concourse.tile as tile
    from concourse import bacc

    tt = _build_tap_tables()
    blocks, E, _tgt, _espc = _build_schedule(tt)
    offs, scs = _build_core_tables(tt, blocks)

    f32 = mybir.dt.float32
    nc = bacc.Bacc("TRN2", target_bir_lowering=False, debug=False,
                   num_devices=NCORES)
    aps = {
        'xb': nc.dram_tensor("xb", [E, 2, C, W], f32,
                             kind="ExternalInput").ap(),
        'sc': nc.dram_tensor("sc", [128, NROW * NSLOT + NSPEC], f32,
                             kind="ExternalInput").ap(),
        'w2': nc.dram_tensor("w2", [128, K * O], f32,
                             kind="ExternalInput").ap(),
        'blkid': nc.dram_tensor("blkid", [1, 1], mybir.dt.int32,
                                kind="ExternalInput").ap(),
        'coefr': nc.dram_tensor("coefr", [128, NSPEC * W], f32,
                                kind="ExternalInput").ap(),
        'biasd': nc.dram_tensor("biasd", [O, 1], f32,
                                kind="ExternalInput").ap(),
        'out': nc.dram_tensor("out", [NROW, O, W], f32,
                              kind="ExternalOutput").ap(),
    }
    with tile.TileContext(nc) as tc:
        _emit_kernel(tc, aps, tt, blocks)
    nc.finalize()

    _CACHE['prog'] = (nc, tt, blocks, E, offs, scs)
    return _CACHE['prog']


def _core_inputs(x, weight, bias, tt, blocks, E, offs, scs):
    """Assemble per-core in_maps. Core c = batch (c // 4), band (c % 4)."""
    w3 = weight.reshape(O, C, K)
    w2k = np.empty((K, 128, O), np.float32)
    for k in range(K):
        w2k[k, :C] = w3[:, :, k].T
        w2k[k, C:] = w3[:, :, k].T
    # [128, K*O] layout: column block k holds W2_k
    w2 = np.ascontiguousarray(w2k.transpose(1, 0, 2).reshape(128, K * O))
    biasd = np.ascontiguousarray(bias.reshape(O, 1).astype(np.float32))

    Gam = tt['Gam'].astype(np.float32)
    coef_on = np.ascontiguousarray(
        np.broadcast_to(Gam[:, None, :], (NSPEC, 128, W))
        .transpose(1, 0, 2).reshape(128, NSPEC * W))
    coef_off = np.zeros((128, NSPEC * W), np.float32)

    in_maps = []
    for cid in range(NCORES):
        b, blk = cid // 4, cid % 4
        xz = np.concatenate([x[b], np.zeros((C, 1, W), np.float32)], axis=1)
        rows = np.asarray(blocks[blk]['events'], np.int64)
        pair_idx = np.stack([rows, rows + 1], axis=1)       # [E, 2]
        xbv = xz[:, pair_idx, :]                            # [C, E, 2, W]
        xbv = np.ascontiguousarray(xbv.transpose(1, 2, 0, 3))  # [E,2,C,W]
        in_maps.append({
            'xb': xbv,
            'sc': scs[blk],
            'w2': w2,
            'blkid': np.array([[blk]], np.int32),
            'coefr': coef_on if blk == 0 else coef_off,
            'biasd': biasd,
        })
    return in_maps


def kernel(x, weight, bias):
    from concourse.bass_utils import run_bass_kernel_spmd
    x = np.asarray(x, dtype=np.float32)
    weight = np.asarray(weight, dtype=np.float32)
    bias = np.asarray(bias, dtype=np.float32)

    nc, tt, blocks, E, offs, scs = _get_compiled()
    in_maps = _core_inputs(x, weight, bias, tt, blocks, E, offs, scs)
    res = run_bass_kernel_spmd(nc, in_maps, core_ids=list(range(NCORES)))

    out = np.empty((B, O, H, W), np.float32)
    for cid in range(NCORES):
        b, blk = cid // 4, cid % 4
        oc = res.results[cid]['out']                        # [NROW, O, W]
        out[b, :, blk * NROW:(blk + 1) * NROW, :] = oc.transpose(1, 0, 2)
    return out


# revision 7
# speedup vs baseline: 3.4663x; 3.4663x over previous
"""Trainium2 Bass kernel for nn_EquiConv2d (equirectangular deformable conv).

Key structural facts exploited (derived from the reference geometry):
  * off_y is exactly longitude-invariant, so each (tap k, row h) samples two
    fixed input rows (iy0, iy0+1) with a constant y-fraction.
  * off_x is longitude-invariant up to the 2*pi wrap: sampling along a row is
    a CIRCULAR shift by a constant s0(k,h) plus a constant x-fraction.
  * Hence the whole deformable conv is 18 matmuls per output row
    ([128=(c x row-pair) contraction, 512 free]) reading circularly
    duplicated row-pair tiles at per-(k,h) column offsets, with the bilinear
    corner weights folded into the stationary (weight) operand.
  * The per-(k,h) column offsets are per-core data: loaded into PE registers
    from an int32 table and applied as dynamic AP slices, so all 8 cores run
    ONE SPMD program.
  * Two fp32 oddities handled exactly: tap (k=7,h=255) is identically zero
    (py==256.0 -> all corners invalid) and tap (k=1,h=1) samples near the
    antipode with fp32-noise-scattered positions -> handled by 3 extra
    matmul slots with per-column coefficient vectors (data-driven, active
    only on the cores owning global row 1).

Sharding: 8 cores = 2 batches x 4 bands of 64 output rows.
"""

import math

import numpy as np

# ----------------------------------------------------------------------------
# problem constants
B, C, H, W = 2, 64, 256, 512
O, KH, KW = 64, 3, 3
K = KH * KW
NCORES = 8
NROW = 64            # output rows per core
NSLOT = 2 * K        # standard matmul slots per row
NSPEC = 3            # special (antipode) slots, accumulated into local row 1
RING = 16            # staged row-pair ring slots
PF = 3               # staging prefetch lead (rows)
SLOTW = 2048         # F(1024) + G(1024) columns per ring slot
GOFF = 1024

_CACHE = {}


# ----------------------------------------------------------------------------
# host-side geometry tables (must replicate reference fp32 semantics exactly)

def _compute_offsets_jax():
    """Bit-exact replica of reference.equi_offsets on jax CPU."""
    import jax
    import jax.numpy as jnp
    cpu = jax.devices("cpu")[0]
    with jax.default_device(cpu):
        dtype = jnp.float32
        pano_H, pano_W, kH, kW = H, W, KH, KW
        Kk = kH * kW
        u = jnp.arange(pano_W, dtype=dtype)
        v = jnp.arange(pano_H, dtype=dtype)
        phi = (u - pano_W / 2.0) / pano_W * (2.0 * math.pi)
        theta = -(v - pano_H / 2.0) / pano_H * math.pi
        cp, sp = jnp.cos(phi), jnp.sin(phi)
        z, one = jnp.zeros_like(cp), jnp.ones_like(cp)
        Ry = jnp.stack([jnp.stack([cp, z, sp], -1),
                        jnp.stack([z, one, z], -1),
                        jnp.stack([-sp, z, cp], -1)], -2)
        ct, st = jnp.cos(theta), jnp.sin(theta)
        zh, oh = jnp.zeros_like(ct), jnp.ones_like(ct)
        Rx = jnp.stack([jnp.stack([oh, zh, zh], -1),
                        jnp.stack([zh, ct, -st], -1),
                        jnp.stack([zh, st, ct], -1)], -2)
        ROT = jnp.einsum('wij,hjk->hwik', Ry, Rx)
        fov_w = kW * (2.0 * math.pi / pano_W)
        focal = (kW / 2.0) / math.tan(fov_w / 2.0)
        hg = (jnp.arange(kH, dtype=dtype)[:, None] + 0.5 - kH / 2.0)
        wg = (jnp.arange(kW, dtype=dtype)[None, :] + 0.5 - kW / 2.0)
        hg = jnp.broadcast_to(hg, (kH, kW)).reshape(Kk)
        wg = jnp.broadcast_to(wg, (kH, kW)).reshape(Kk)
        rays0 = jnp.stack([wg / focal, hg / focal, jnp.ones(Kk, dtype)], 0)
        rays0 = rays0 / jnp.linalg.norm(rays0, axis=0, keepdims=True)
        rays = jnp.einsum('hwik,kn->hwin', ROT, rays0)
        phi2 = jnp.arctan2(rays[..., 0, :], rays[..., 2, :])
        th2 = jnp.arcsin(jnp.clip(rays[..., 1, :], -1.0, 1.0))
        x = pano_W / (2.0 * math.pi) * phi2 + pano_W / 2.0
        y = pano_H / math.pi * th2 + pano_H / 2.0
        off_x = x - (wg[None, None, :] + u[None, :, None])
        off_y = y - (hg[None, None, :] + v[:, None, None])
        return (np.asarray(jnp.transpose(off_y, (2, 0, 1))),
                np.asarray(jnp.transpose(off_x, (2, 0, 1))))


def _build_tap_tables():
    off_y, off_x = _compute_offsets_jax()
    ky = np.repeat(np.arange(KH), KW).astype(np.float32)
    kx = np.tile(np.arange(KW), KH).astype(np.float32)
    base_x = (np.arange(W, dtype=np.float32) - np.float32(1))
    base_y = (np.arange(H, dtype=np.float32) - np.float32(1))
    px = (base_x[None, None, :] + kx[:, None, None] + off_x).astype(np.float32)
    py = (base_y[None, :, None] + ky[:, None, None] + off_y).astype(np.float32)
    pyc = py[:, :, 0]
    assert np.all(py == pyc[:, :, None]), "off_y not longitude-invariant"

    iy0 = np.floor(pyc).astype(np.int64)
    wy1 = (pyc - np.floor(pyc)).astype(np.float64)
    v0 = (iy0 >= 0) & (iy0 < H)
    v1 = (iy0 + 1 >= 0) & (iy0 + 1 < H)
    cy0 = np.where(v0, 1.0 - wy1, 0.0)
    cy1 = np.where(v1, wy1, 0.0)

    Draw = np.mod((px.astype(np.float64) - np.arange(W)[None, None, :]), 512.0)
    ang = Draw / 512.0 * 2 * np.pi
    mean = np.mod(np.angle(np.exp(1j * ang).mean(axis=2)) / (2 * np.pi) * 512.0,
                  512.0)
    resid = np.mod(Draw - mean[:, :, None] + 256.0, 512.0) - 256.0
    D = mean + np.median(resid, axis=2)
    s0 = np.mod(np.floor(D), 512).astype(np.int64)
    frac = D - np.floor(D)

    special = np.zeros((K, H), dtype=bool)
    special[1, 1] = True
    dead = (cy0 == 0.0) & (cy1 == 0.0)

    Ddev = np.abs(np.mod(Draw - D[:, :, None] + 256.0, 512.0) - 256.0)
    dev = Ddev.max(axis=2)
    bad = (dev > 5e-4) & ~special & ~dead
    assert not bad.any(), f"unrepresentable taps: {np.argwhere(bad)}"

    def ref_coefs(p):
        x0 = math.floor(p)
        fr = p - x0
        out = {}
        for ix, wt in ((x0, 1.0 - fr), (x0 + 1, fr)):
            if 0 <= ix < W and wt != 0.0:
                out[ix] = out.get(ix, 0.0) + wt
        return out

    # seam variant selection: decided by the exact fp32 px at the wrap column
    slot0_useG = np.zeros((K, H), dtype=bool)
    slot1_useF = np.zeros((K, H), dtype=bool)
    for k in range(K):
        for h in range(H):
            if special[k, h] or dead[k, h]:
                continue
            s = int(s0[k, h]); fr = frac[k, h]
            if s >= 1:
                w0 = (512 - s) % 512
                rc = ref_coefs(float(px[k, h, w0]))
                slot0_useG[k, h] = (abs(rc.get(0, 0.0))
                                    < abs(rc.get(0, 0.0) - (1 - fr)))
            w1 = (511 - s) % 512
            rc = ref_coefs(float(px[k, h, w1]))
            slot1_useF[k, h] = (abs(rc.get(0, 0.0) - fr)
                                < abs(rc.get(0, 0.0)))

    # special tap (1,1): per-column coefficients on F offsets 255..257
    pxs = px[1, 1, :].astype(np.float64)
    Gam = np.zeros((3, W), dtype=np.float64)
    for w in range(W):
        p = pxs[w]
        x0 = math.floor(p)
        fr = p - x0
        for ix, wt in ((x0, 1.0 - fr), (x0 + 1, fr)):
            if 0 <= ix < W and wt != 0.0:
                found = False
                for jj in range(3):
                    if (255 + jj + w) % 512 == ix % 512:
                        Gam[jj, w] += wt
                        found = True
                        break
                assert found, (w, p, ix)

    return dict(iy0=iy0, cy0=cy0, cy1=cy1, s0=s0, frac=frac,
                slot0_useG=slot0_useG, slot1_useF=slot1_useF,
                special=special, dead=dead, Gam=Gam)


# ----------------------------------------------------------------------------
# uniform SPMD schedule

def _build_schedule(tt):
    blocks = []
    for blk in range(4):
        h0 = blk * NROW
        ev_of, events, first_use = {}, [], []
        need = np.zeros((NROW, K), np.int64)
        for lh in range(NROW):
            for k in range(K):
                r = int(np.clip(tt['iy0'][k, h0 + lh], 0, 255))
                if r not in ev_of:
                    ev_of[r] = len(events)
                    events.append(r)
                    first_use.append(lh)
                need[lh, k] = ev_of[r]
        blocks.append(dict(events=events, first_use=first_use, need=need))

    E = max(len(b['events']) for b in blocks)
    for b in blocks:
        while len(b['events']) < E:
            b['events'].append(b['events'][-1])

    # uniform staged-count before row lh:  tgt(lh) = U[min(lh+PF, NROW-1)]
    U = np.zeros(NROW, np.int64)
    for lh in range(NROW):
        U[lh] = max(int(np.searchsorted(np.asarray(b['first_use']), lh, 'right'))
                    for b in blocks)
    tgt = np.array([U[min(lh + PF, NROW - 1)] for lh in range(NROW)])

    # ring-overwrite feasibility
    ls = np.full(E, NROW, np.int64)
    for e in range(E):
        hit = np.where(tgt > e)[0]
        if len(hit):
            ls[e] = hit[0]
    for b in blocks:
        lastuse = {}
        for lh in range(NROW):
            for k in range(K):
                lastuse[int(b['need'][lh, k])] = lh
        for e in range(RING, E):
            prev = e - RING
            if prev in lastuse:
                assert lastuse[prev] < ls[e], \
                    f"RING={RING} too small: ev{e} overwrites ev{prev} " \
                    f"(lastuse {lastuse[prev]}, staged before row {ls[e]})"
    espc = int(blocks[0]['need'][1, 1])
    return blocks, E, tgt, espc


def _build_scale_tables(tt):
    """Per-block fp64 scale vectors [NROW, NSLOT, 128] (geometry only)."""
    scs = []
    for blk in range(4):
        h0 = blk * NROW
        sc = np.zeros((NROW, NSLOT, 128), np.float64)
        for lh in range(NROW):
            h = h0 + lh
            for k in range(K):
                if tt['dead'][k, h] or tt['special'][k, h]:
                    continue
                fr = tt['frac'][k, h]
                c0, c1 = tt['cy0'][k, h], tt['cy1'][k, h]
                sc[lh, 2 * k, :64] = c0 * (1 - fr)
                sc[lh, 2 * k, 64:] = c1 * (1 - fr)
                sc[lh, 2 * k + 1, :64] = c0 * fr
                sc[lh, 2 * k + 1, 64:] = c1 * fr
        scs.append(sc)
    return scs


# ----------------------------------------------------------------------------
# device program

def _emit_section(tc, aps, tiles, tt, blkinfo, j):
    """Emit one per-band section (all-static APs)."""
    import concourse.mybir as mybir
    nc = tc.nc
    f16 = mybir.dt.float16
    f32 = mybir.dt.float32
    buf, coeft, biast, ltst = tiles
    xb, outd, lt = aps['xb'], aps['out'], aps['lt']
    need = blkinfo['need']
    first_use = blkinfo['first_use']
    E_j = len(first_use)

    cum = [int(np.searchsorted(np.asarray(first_use), lh, 'right'))
           for lh in range(NROW)]
    tgt = [cum[min(lh + PF, NROW - 1)] for lh in range(NROW)]

    ls = [NROW] * E_j
    for e in range(E_j):
        for lh in range(NROW):
            if tgt[lh] > e:
                ls[e] = lh
                break
    lastuse = {}
    for lh in range(NROW):
        for k in range(K):
            lastuse[int(need[lh, k])] = lh
    for e in range(RING, E_j):
        if e - RING in lastuse:
            assert lastuse[e - RING] < ls[e], (j, e)

    def stage(e):
        base = (e % RING) * SLOTW
        src = xb[e].rearrange("p c w -> (p c) w")
        nc.sync.dma_start(buf[:, base:base + W], src)
        nc.sync.dma_start(buf[:, base + W:base + 2 * W], src)
        nc.vector.tensor_copy(buf[:, base + GOFF:base + GOFF + W],
                              buf[:, base + 1:base + 1 + W])
        nc.scalar.copy(buf[:, base + GOFF + W:base + GOFF + 2 * W],
                       buf[:, base + 1:base + 1 + W])
        nc.gpsimd.memset(buf[:, base + GOFF + 511:base + GOFF + 512], 0.0)

    psp, ltp, zp, outp = tiles_pools[0]

    staged = 0
    for lh in range(NROW):
        while staged < tgt[lh]:
            stage(staged)
            staged += 1
        h = j * NROW + lh
        ltt = ltp.tile([128, NSLOT * O], f16, tag="ltt")
        nc.sync.dma_start(ltt, lt[lh])
        ps = psp.tile([O, W], f32, tag="ps")
        nmm = NSLOT + (NSPEC if (j == 0 and lh == 1) else 0)
        mi = 0
        for k in range(K):
            base = int(need[lh, k] % RING) * SLOTW
            s = int(tt['s0'][k, h])
            if tt['dead'][k, h] or tt['special'][k, h]:
                v0 = v1 = base
            else:
                if tt['slot0_useG'][k, h] and s >= 1:
                    v0 = base + GOFF + s - 1
                else:
                    v0 = base + s
                v1 = base + s + 1 if tt['slot1_useF'][k, h] \
                    else base + GOFF + s
            for sl, v in ((2 * k, v0), (2 * k + 1, v1)):
                nc.tensor.matmul(ps, ltt[:, sl * O:(sl + 1) * O],
                                 buf[:, v:v + W],
                                 start=(mi == 0), stop=(mi == nmm - 1))
                mi += 1
        if j == 0 and lh == 1:
            sbase = int(need[1, 1] % RING) * SLOTW
            for jj in range(NSPEC):
                zt = zp.tile([128, W], f16, tag="spz")
                nc.vector.tensor_mul(
                    zt, buf[:, sbase + 255 + jj:sbase + 255 + jj + W],
                    coeft[:, jj * W:(jj + 1) * W])
                nc.tensor.matmul(ps, ltst[:, jj * O:(jj + 1) * O], zt,
                                 start=False, stop=(mi == nmm - 1))
                mi += 1
        ot = outp.tile([O, W], f32, tag="out")
        nc.scalar.activation(ot, ps,
                             mybir.ActivationFunctionType.Identity,
                             bias=biast, scale=1.0)
        nc.sync.dma_start(outd[lh], ot)


tiles_pools = [None]


def _emit_kernel(tc, aps, tt, blocks):
    import concourse.mybir as mybir
    nc = tc.nc
    f16 = mybir.dt.float16
    f32 = mybir.dt.float32

    with tc.tile_pool(name="bigp", bufs=1) as bigp, \
         tc.tile_pool(name="ltp", bufs=3) as ltp, \
         tc.tile_pool(name="zp", bufs=3) as zp, \
         tc.tile_pool(name="psp", bufs=4, space="PSUM") as psp, \
         tc.tile_pool(name="outp", bufs=3) as outp:

        buf = bigp.tile([128, RING * SLOTW], f16)
        coeft = bigp.tile([128, NSPEC * W], f16)
        biast = bigp.tile([O, 1], f32)
        ltst = bigp.tile([128, NSPEC * O], f16)

        nc.sync.dma_start(coeft, aps['coefr'])
        nc.sync.dma_start(biast, aps['biasd'])
        nc.sync.dma_start(ltst, aps['lts'])

        blkv = nc.values_load(aps['blkid'][0:1, 0:1],
                              min_val=0, max_val=3,
                              skip_runtime_bounds_check=True)

        tiles = (buf, coeft, biast, ltst)
        tiles_pools[0] = (psp, ltp, zp, outp)
        for j in range(4):
            with tc.If(blkv == j):
                _emit_section(tc, aps, tiles, tt, blocks[j], j)


def _get_compiled():
    """Build tables, schedule, and the Bass program once."""
    if 'prog' in _CACHE:
        return _CACHE['prog']
    import concourse.mybir as mybir
    import concourse.tile as tile
    from concourse import bacc

    tt = _build_tap_tables()
    blocks, E, _tgt, _espc = _build_schedule(tt)
    scs = _build_scale_tables(tt)

    f16 = mybir.dt.float16
    f32 = mybir.dt.float32
    nc = bacc.Bacc("TRN2", target_bir_lowering=False, debug=False,
                   num_devices=NCORES)
    aps = {
        'xb': nc.dram_tensor("xb", [E, 2, C, W], f16,
                             kind="ExternalInput").ap(),
        'lt': nc.dram_tensor("lt", [NROW, 128, NSLOT * O], f16,
                             kind="ExternalInput").ap(),
        'lts': nc.dram_tensor("lts", [128, NSPEC * O], f16,
                              kind="ExternalInput").ap(),
        'blkid': nc.dram_tensor("blkid", [1, 1], mybir.dt.int32,
                                kind="ExternalInput").ap(),
        'coefr': nc.dram_tensor("coefr", [128, NSPEC * W], f16,
                                kind="ExternalInput").ap(),
        'biasd': nc.dram_tensor("biasd", [O, 1], f32,
                                kind="ExternalInput").ap(),
        'out': nc.dram_tensor("out", [NROW, O, W], f32,
                              kind="ExternalOutput").ap(),
    }
    with tile.TileContext(nc) as tc:
        _emit_kernel(tc, aps, tt, blocks)
    nc.finalize()

    _CACHE['prog'] = (nc, tt, blocks, E, scs)
    return _CACHE['prog']


def _core_inputs(x, weight, bias, tt, blocks, E, scs):
    """Assemble per-core in_maps. Core c = batch (c // 4), band (c % 4)."""
    w3 = weight.reshape(O, C, K).astype(np.float64)
    # W2d[p, k, o]: channel-duplicated weights on the contraction axis
    w2d = np.empty((128, K, O), np.float64)
    w2d[:C] = w3.transpose(1, 2, 0)
    w2d[C:] = w3.transpose(1, 2, 0)
    # slot-expanded: [NSLOT, 128, O]
    w2s = np.repeat(w2d.transpose(1, 0, 2), 2, axis=0)
    biasd = np.ascontiguousarray(bias.reshape(O, 1).astype(np.float32))

    lts_on = np.zeros((128, NSPEC * O), np.float16)
    for jj in range(NSPEC):
        lts_on[:C, jj * O:(jj + 1) * O] = w2d[:C, 1, :].astype(np.float16)
    lts_off = np.zeros((128, NSPEC * O), np.float16)

    Gam = tt['Gam'].astype(np.float16)
    coef_on = np.ascontiguousarray(
        np.broadcast_to(Gam[:, None, :], (NSPEC, 128, W))
        .transpose(1, 0, 2).reshape(128, NSPEC * W))
    coef_off = np.zeros((128, NSPEC * W), np.float16)

    lt_blk = []
    for blk in range(4):
        # [l, s, p] x [s, p, o] -> [l, p, s, o]
        ltv = np.einsum('lsp,spo->lpso', scs[blk], w2s)
        lt_blk.append(np.ascontiguousarray(
            ltv.reshape(NROW, 128, NSLOT * O)).astype(np.float16))

    in_maps = []
    for cid in range(NCORES):
        b, blk = cid // 4, cid % 4
        xz = np.concatenate([x[b], np.zeros((C, 1, W), x.dtype)], axis=1)
        xz = xz.astype(np.float16)
        rows = np.asarray(blocks[blk]['events'], np.int64)
        pair_idx = np.stack([rows, rows + 1], axis=1)       # [E, 2]
        xbv = xz[:, pair_idx, :]                            # [C, E, 2, W]
        xbv = np.ascontiguousarray(xbv.transpose(1, 2, 0, 3))  # [E,2,C,W]
        in_maps.append({
            'xb': xbv,
            'lt': lt_blk[blk],
            'lts': lts_on if blk == 0 else lts_off,
            'blkid': np.array([[blk]], np.int32),
            'coefr': coef_on if blk == 0 else coef_off,
            'biasd': biasd,
        })
    return in_maps


def kernel(x, weight, bias):
    from concourse.bass_utils import run_bass_kernel_spmd
    x = np.asarray(x, dtype=np.float32)
    weight = np.asarray(weight, dtype=np.float32)
    bias = np.asarray(bias, dtype=np.float32)

    nc, tt, blocks, E, scs = _get_compiled()
    in_maps = _core_inputs(x, weight, bias, tt, blocks, E, scs)
    res = run_bass_kernel_spmd(nc, in_maps, core_ids=list(range(NCORES)))

    out = np.empty((B, O, H, W), np.float32)
    for cid in range(NCORES):
        b, blk = cid // 4, cid % 4
        oc = res.results[cid]['out']                        # [NROW, O, W]
        out[b, :, blk * NROW:(blk + 1) * NROW, :] = oc.transpose(1, 0, 2)
    return out


# revision 8
# speedup vs baseline: 4.0386x; 1.1651x over previous
"""Trainium2 Bass kernel for nn_EquiConv2d (equirectangular deformable conv).

Key structural facts exploited (derived from the reference geometry):
  * off_y is exactly longitude-invariant, so each (tap k, row h) samples two
    fixed input rows (iy0, iy0+1) with a constant y-fraction.
  * off_x is longitude-invariant up to the 2*pi wrap: sampling along a row is
    a CIRCULAR shift by a constant s0(k,h) plus a constant x-fraction.
  * Hence the whole deformable conv is 18 matmuls per output row
    ([128=(c x row-pair) contraction, 512 free]) reading circularly
    duplicated row-pair tiles at per-(k,h) column offsets, with the bilinear
    corner weights folded into the stationary (weight) operand.
  * The per-(k,h) column offsets are per-core data: loaded into PE registers
    from an int32 table and applied as dynamic AP slices, so all 8 cores run
    ONE SPMD program.
  * Two fp32 oddities handled exactly: tap (k=7,h=255) is identically zero
    (py==256.0 -> all corners invalid) and tap (k=1,h=1) samples near the
    antipode with fp32-noise-scattered positions -> handled by 3 extra
    matmul slots with per-column coefficient vectors (data-driven, active
    only on the cores owning global row 1).

Sharding: 8 cores = 2 batches x 4 bands of 64 output rows.
"""

import math

import numpy as np

# ----------------------------------------------------------------------------
# problem constants
B, C, H, W = 2, 64, 256, 512
O, KH, KW = 64, 3, 3
K = KH * KW
NCORES = 8
NROW = 64            # output rows per core
NSLOT = 2 * K        # standard matmul slots per row
NSPEC = 3            # special (antipode) slots, accumulated into local row 1
RING = 16            # staged row-pair ring slots
PF = 3               # staging prefetch lead (rows)
SLOTW = 2048         # F(1024) + G(1024) columns per ring slot
GOFF = 1024
SKIP_TOL = 1e-4       # drop matmul slots with |weight| below this

_CACHE = {}


# ----------------------------------------------------------------------------
# host-side geometry tables (must replicate reference fp32 semantics exactly)

def _compute_offsets_jax():
    """Bit-exact replica of reference.equi_offsets on jax CPU."""
    import jax
    import jax.numpy as jnp
    cpu = jax.devices("cpu")[0]
    with jax.default_device(cpu):
        dtype = jnp.float32
        pano_H, pano_W, kH, kW = H, W, KH, KW
        Kk = kH * kW
        u = jnp.arange(pano_W, dtype=dtype)
        v = jnp.arange(pano_H, dtype=dtype)
        phi = (u - pano_W / 2.0) / pano_W * (2.0 * math.pi)
        theta = -(v - pano_H / 2.0) / pano_H * math.pi
        cp, sp = jnp.cos(phi), jnp.sin(phi)
        z, one = jnp.zeros_like(cp), jnp.ones_like(cp)
        Ry = jnp.stack([jnp.stack([cp, z, sp], -1),
                        jnp.stack([z, one, z], -1),
                        jnp.stack([-sp, z, cp], -1)], -2)
        ct, st = jnp.cos(theta), jnp.sin(theta)
        zh, oh = jnp.zeros_like(ct), jnp.ones_like(ct)
        Rx = jnp.stack([jnp.stack([oh, zh, zh], -1),
                        jnp.stack([zh, ct, -st], -1),
                        jnp.stack([zh, st, ct], -1)], -2)
        ROT = jnp.einsum('wij,hjk->hwik', Ry, Rx)
        fov_w = kW * (2.0 * math.pi / pano_W)
        focal = (kW / 2.0) / math.tan(fov_w / 2.0)
        hg = (jnp.arange(kH, dtype=dtype)[:, None] + 0.5 - kH / 2.0)
        wg = (jnp.arange(kW, dtype=dtype)[None, :] + 0.5 - kW / 2.0)
        hg = jnp.broadcast_to(hg, (kH, kW)).reshape(Kk)
        wg = jnp.broadcast_to(wg, (kH, kW)).reshape(Kk)
        rays0 = jnp.stack([wg / focal, hg / focal, jnp.ones(Kk, dtype)], 0)
        rays0 = rays0 / jnp.linalg.norm(rays0, axis=0, keepdims=True)
        rays = jnp.einsum('hwik,kn->hwin', ROT, rays0)
        phi2 = jnp.arctan2(rays[..., 0, :], rays[..., 2, :])
        th2 = jnp.arcsin(jnp.clip(rays[..., 1, :], -1.0, 1.0))
        x = pano_W / (2.0 * math.pi) * phi2 + pano_W / 2.0
        y = pano_H / math.pi * th2 + pano_H / 2.0
        off_x = x - (wg[None, None, :] + u[None, :, None])
        off_y = y - (hg[None, None, :] + v[:, None, None])
        return (np.asarray(jnp.transpose(off_y, (2, 0, 1))),
                np.asarray(jnp.transpose(off_x, (2, 0, 1))))


def _build_tap_tables():
    off_y, off_x = _compute_offsets_jax()
    ky = np.repeat(np.arange(KH), KW).astype(np.float32)
    kx = np.tile(np.arange(KW), KH).astype(np.float32)
    base_x = (np.arange(W, dtype=np.float32) - np.float32(1))
    base_y = (np.arange(H, dtype=np.float32) - np.float32(1))
    px = (base_x[None, None, :] + kx[:, None, None] + off_x).astype(np.float32)
    py = (base_y[None, :, None] + ky[:, None, None] + off_y).astype(np.float32)
    pyc = py[:, :, 0]
    assert np.all(py == pyc[:, :, None]), "off_y not longitude-invariant"

    iy0 = np.floor(pyc).astype(np.int64)
    wy1 = (pyc - np.floor(pyc)).astype(np.float64)
    v0 = (iy0 >= 0) & (iy0 < H)
    v1 = (iy0 + 1 >= 0) & (iy0 + 1 < H)
    cy0 = np.where(v0, 1.0 - wy1, 0.0)
    cy1 = np.where(v1, wy1, 0.0)

    Draw = np.mod((px.astype(np.float64) - np.arange(W)[None, None, :]), 512.0)
    ang = Draw / 512.0 * 2 * np.pi
    mean = np.mod(np.angle(np.exp(1j * ang).mean(axis=2)) / (2 * np.pi) * 512.0,
                  512.0)
    resid = np.mod(Draw - mean[:, :, None] + 256.0, 512.0) - 256.0
    D = mean + np.median(resid, axis=2)
    s0 = np.mod(np.floor(D), 512).astype(np.int64)
    frac = D - np.floor(D)

    special = np.zeros((K, H), dtype=bool)
    special[1, 1] = True
    dead = (cy0 == 0.0) & (cy1 == 0.0)

    Ddev = np.abs(np.mod(Draw - D[:, :, None] + 256.0, 512.0) - 256.0)
    dev = Ddev.max(axis=2)
    bad = (dev > 5e-4) & ~special & ~dead
    assert not bad.any(), f"unrepresentable taps: {np.argwhere(bad)}"

    def ref_coefs(p):
        x0 = math.floor(p)
        fr = p - x0
        out = {}
        for ix, wt in ((x0, 1.0 - fr), (x0 + 1, fr)):
            if 0 <= ix < W and wt != 0.0:
                out[ix] = out.get(ix, 0.0) + wt
        return out

    # seam variant selection: decided by the exact fp32 px at the wrap column
    slot0_useG = np.zeros((K, H), dtype=bool)
    slot1_useF = np.zeros((K, H), dtype=bool)
    for k in range(K):
        for h in range(H):
            if special[k, h] or dead[k, h]:
                continue
            s = int(s0[k, h]); fr = frac[k, h]
            if s >= 1:
                w0 = (512 - s) % 512
                rc = ref_coefs(float(px[k, h, w0]))
                slot0_useG[k, h] = (abs(rc.get(0, 0.0))
                                    < abs(rc.get(0, 0.0) - (1 - fr)))
            w1 = (511 - s) % 512
            rc = ref_coefs(float(px[k, h, w1]))
            slot1_useF[k, h] = (abs(rc.get(0, 0.0) - fr)
                                < abs(rc.get(0, 0.0)))

    # special tap (1,1): per-column coefficients on F offsets 255..257
    pxs = px[1, 1, :].astype(np.float64)
    Gam = np.zeros((3, W), dtype=np.float64)
    for w in range(W):
        p = pxs[w]
        x0 = math.floor(p)
        fr = p - x0
        for ix, wt in ((x0, 1.0 - fr), (x0 + 1, fr)):
            if 0 <= ix < W and wt != 0.0:
                found = False
                for jj in range(3):
                    if (255 + jj + w) % 512 == ix % 512:
                        Gam[jj, w] += wt
                        found = True
                        break
                assert found, (w, p, ix)

    return dict(iy0=iy0, cy0=cy0, cy1=cy1, s0=s0, frac=frac,
                slot0_useG=slot0_useG, slot1_useF=slot1_useF,
                special=special, dead=dead, Gam=Gam)


# ----------------------------------------------------------------------------
# uniform SPMD schedule

def _build_schedule(tt):
    blocks = []
    for blk in range(4):
        h0 = blk * NROW
        ev_of, events, first_use = {}, [], []
        need = np.zeros((NROW, K), np.int64)
        for lh in range(NROW):
            for k in range(K):
                r = int(np.clip(tt['iy0'][k, h0 + lh], 0, 255))
                if r not in ev_of:
                    ev_of[r] = len(events)
                    events.append(r)
                    first_use.append(lh)
                need[lh, k] = ev_of[r]
        blocks.append(dict(events=events, first_use=first_use, need=need))

    E = max(len(b['events']) for b in blocks)
    for b in blocks:
        while len(b['events']) < E:
            b['events'].append(b['events'][-1])

    # uniform staged-count before row lh:  tgt(lh) = U[min(lh+PF, NROW-1)]
    U = np.zeros(NROW, np.int64)
    for lh in range(NROW):
        U[lh] = max(int(np.searchsorted(np.asarray(b['first_use']), lh, 'right'))
                    for b in blocks)
    tgt = np.array([U[min(lh + PF, NROW - 1)] for lh in range(NROW)])

    # ring-overwrite feasibility
    ls = np.full(E, NROW, np.int64)
    for e in range(E):
        hit = np.where(tgt > e)[0]
        if len(hit):
            ls[e] = hit[0]
    for b in blocks:
        lastuse = {}
        for lh in range(NROW):
            for k in range(K):
                lastuse[int(b['need'][lh, k])] = lh
        for e in range(RING, E):
            prev = e - RING
            if prev in lastuse:
                assert lastuse[prev] < ls[e], \
                    f"RING={RING} too small: ev{e} overwrites ev{prev} " \
                    f"(lastuse {lastuse[prev]}, staged before row {ls[e]})"
    espc = int(blocks[0]['need'][1, 1])
    return blocks, E, tgt, espc


def _build_scale_tables(tt):
    """Per-block fp64 scale vectors [NROW, NSLOT, 128] (geometry only)."""
    scs = []
    for blk in range(4):
        h0 = blk * NROW
        sc = np.zeros((NROW, NSLOT, 128), np.float64)
        for lh in range(NROW):
            h = h0 + lh
            for k in range(K):
                if tt['dead'][k, h] or tt['special'][k, h]:
                    continue
                fr = tt['frac'][k, h]
                c0, c1 = tt['cy0'][k, h], tt['cy1'][k, h]
                sc[lh, 2 * k, :64] = c0 * (1 - fr)
                sc[lh, 2 * k, 64:] = c1 * (1 - fr)
                sc[lh, 2 * k + 1, :64] = c0 * fr
                sc[lh, 2 * k + 1, 64:] = c1 * fr
        scs.append(sc)
    return scs


# ----------------------------------------------------------------------------
# device program

def _emit_section(tc, aps, tiles, tt, blkinfo, j):
    """Emit one per-band section (all-static APs)."""
    import concourse.mybir as mybir
    nc = tc.nc
    f16 = mybir.dt.float16
    f32 = mybir.dt.float32
    buf, coeft, biast, ltst = tiles
    xb, outd, lt = aps['xb'], aps['out'], aps['lt']
    need = blkinfo['need']
    first_use = blkinfo['first_use']
    E_j = len(first_use)

    cum = [int(np.searchsorted(np.asarray(first_use), lh, 'right'))
           for lh in range(NROW)]
    tgt = [cum[min(lh + PF, NROW - 1)] for lh in range(NROW)]

    ls = [NROW] * E_j
    for e in range(E_j):
        for lh in range(NROW):
            if tgt[lh] > e:
                ls[e] = lh
                break
    lastuse = {}
    for lh in range(NROW):
        for k in range(K):
            lastuse[int(need[lh, k])] = lh
    for e in range(RING, E_j):
        if e - RING in lastuse:
            assert lastuse[e - RING] < ls[e], (j, e)

    def stage(e):
        base = (e % RING) * SLOTW
        src = xb[e].rearrange("p c w -> (p c) w")
        nc.sync.dma_start(buf[:, base:base + W], src)
        nc.sync.dma_start(buf[:, base + W:base + 2 * W], src)
        nc.vector.tensor_copy(buf[:, base + GOFF:base + GOFF + W],
                              buf[:, base + 1:base + 1 + W])
        nc.scalar.copy(buf[:, base + GOFF + W:base + GOFF + 2 * W],
                       buf[:, base + 1:base + 1 + W])
        nc.gpsimd.memset(buf[:, base + GOFF + 511:base + GOFF + 512], 0.0)

    psp, ltp, zp, outp = tiles_pools[0]

    staged = 0
    for lh in range(NROW):
        while staged < tgt[lh]:
            stage(staged)
            staged += 1
        h = j * NROW + lh
        ltt = ltp.tile([128, NSLOT * O], f16, tag="ltt")
        nc.sync.dma_start(ltt, lt[lh])
        ps = psp.tile([O, W], f32, tag="ps")
        # collect (slot, rhs-offset) slots, statically skipping near-zero
        # weights: slot1 weight=frac, slot0 weight=1-frac; drop < SKIP_TOL
        emits = []
        for k in range(K):
            base = int(need[lh, k] % RING) * SLOTW
            s = int(tt['s0'][k, h])
            if tt['dead'][k, h] or tt['special'][k, h]:
                continue
            fr = float(tt['frac'][k, h])
            if tt['slot0_useG'][k, h] and s >= 1:
                v0 = base + GOFF + s - 1
            else:
                v0 = base + s
            v1 = base + s + 1 if tt['slot1_useF'][k, h] \
                else base + GOFF + s
            if 1.0 - fr >= SKIP_TOL:
                emits.append((2 * k, v0))
            if fr >= SKIP_TOL:
                emits.append((2 * k + 1, v1))
        nmm = len(emits) + (NSPEC if (j == 0 and lh == 1) else 0)
        mi = 0
        for sl, v in emits:
            nc.tensor.matmul(ps, ltt[:, sl * O:(sl + 1) * O],
                             buf[:, v:v + W],
                             start=(mi == 0), stop=(mi == nmm - 1))
            mi += 1
        if j == 0 and lh == 1:
            sbase = int(need[1, 1] % RING) * SLOTW
            for jj in range(NSPEC):
                zt = zp.tile([128, W], f16, tag="spz")
                nc.vector.tensor_mul(
                    zt, buf[:, sbase + 255 + jj:sbase + 255 + jj + W],
                    coeft[:, jj * W:(jj + 1) * W])
                nc.tensor.matmul(ps, ltst[:, jj * O:(jj + 1) * O], zt,
                                 start=False, stop=(mi == nmm - 1))
                mi += 1
        ot = outp.tile([O, W], f32, tag="out")
        nc.scalar.activation(ot, ps,
                             mybir.ActivationFunctionType.Identity,
                             bias=biast, scale=1.0)
        nc.sync.dma_start(outd[lh], ot)


tiles_pools = [None]


def _emit_kernel(tc, aps, tt, blocks):
    import concourse.mybir as mybir
    nc = tc.nc
    f16 = mybir.dt.float16
    f32 = mybir.dt.float32

    with tc.tile_pool(name="bigp", bufs=1) as bigp, \
         tc.tile_pool(name="ltp", bufs=3) as ltp, \
         tc.tile_pool(name="zp", bufs=3) as zp, \
         tc.tile_pool(name="psp", bufs=4, space="PSUM") as psp, \
         tc.tile_pool(name="outp", bufs=3) as outp:

        buf = bigp.tile([128, RING * SLOTW], f16)
        coeft = bigp.tile([128, NSPEC * W], f16)
        biast = bigp.tile([O, 1], f32)
        ltst = bigp.tile([128, NSPEC * O], f16)

        nc.sync.dma_start(coeft, aps['coefr'])
        nc.sync.dma_start(biast, aps['biasd'])
        nc.sync.dma_start(ltst, aps['lts'])

        blkv = nc.values_load(aps['blkid'][0:1, 0:1],
                              min_val=0, max_val=3,
                              skip_runtime_bounds_check=True)

        tiles = (buf, coeft, biast, ltst)
        tiles_pools[0] = (psp, ltp, zp, outp)
        for j in range(4):
            with tc.If(blkv == j):
                _emit_section(tc, aps, tiles, tt, blocks[j], j)


def _get_compiled():
    """Build tables, schedule, and the Bass program once."""
    if 'prog' in _CACHE:
        return _CACHE['prog']
    import concourse.mybir as mybir
    import concourse.tile as tile
    from concourse import bacc

    tt = _build_tap_tables()
    blocks, E, _tgt, _espc = _build_schedule(tt)
    scs = _build_scale_tables(tt)

    f16 = mybir.dt.float16
    f32 = mybir.dt.float32
    nc = bacc.Bacc("TRN2", target_bir_lowering=False, debug=False,
                   num_devices=NCORES)
    aps = {
        'xb': nc.dram_tensor("xb", [E, 2, C, W], f16,
                             kind="ExternalInput").ap(),
        'lt': nc.dram_tensor("lt", [NROW, 128, NSLOT * O], f16,
                             kind="ExternalInput").ap(),
        'lts': nc.dram_tensor("lts", [128, NSPEC * O], f16,
                              kind="ExternalInput").ap(),
        'blkid': nc.dram_tensor("blkid", [1, 1], mybir.dt.int32,
                                kind="ExternalInput").ap(),
        'coefr': nc.dram_tensor("coefr", [128, NSPEC * W], f16,
                                kind="ExternalInput").ap(),
        'biasd': nc.dram_tensor("biasd", [O, 1], f32,
                                kind="ExternalInput").ap(),
        'out': nc.dram_tensor("out", [NROW, O, W], f32,
                              kind="ExternalOutput").ap(),
    }
    with tile.TileContext(nc) as tc:
        _emit_kernel(tc, aps, tt, blocks)
    nc.finalize()

    _CACHE['prog'] = (nc, tt, blocks, E, scs)
    return _CACHE['prog']


def _core_inputs(x, weight, bias, tt, blocks, E, scs):
    """Assemble per-core in_maps. Core c = batch (c // 4), band (c % 4)."""
    w3 = weight.reshape(O, C, K).astype(np.float64)
    # W2d[p, k, o]: channel-duplicated weights on the contraction axis
    w2d = np.empty((128, K, O), np.float64)
    w2d[:C] = w3.transpose(1, 2, 0)
    w2d[C:] = w3.transpose(1, 2, 0)
    # slot-expanded: [NSLOT, 128, O]
    w2s = np.repeat(w2d.transpose(1, 0, 2), 2, axis=0)
    biasd = np.ascontiguousarray(bias.reshape(O, 1).astype(np.float32))

    lts_on = np.zeros((128, NSPEC * O), np.float16)
    for jj in range(NSPEC):
        lts_on[:C, jj * O:(jj + 1) * O] = w2d[:C, 1, :].astype(np.float16)
    lts_off = np.zeros((128, NSPEC * O), np.float16)

    Gam = tt['Gam'].astype(np.float16)
    coef_on = np.ascontiguousarray(
        np.broadcast_to(Gam[:, None, :], (NSPEC, 128, W))
        .transpose(1, 0, 2).reshape(128, NSPEC * W))
    coef_off = np.zeros((128, NSPEC * W), np.float16)

    lt_blk = []
    for blk in range(4):
        # [l, s, p] x [s, p, o] -> [l, p, s, o]
        ltv = np.einsum('lsp,spo->lpso', scs[blk], w2s)
        lt_blk.append(np.ascontiguousarray(
            ltv.reshape(NROW, 128, NSLOT * O)).astype(np.float16))

    in_maps = []
    for cid in range(NCORES):
        b, blk = cid // 4, cid % 4
        xz = np.concatenate([x[b], np.zeros((C, 1, W), x.dtype)], axis=1)
        xz = xz.astype(np.float16)
        rows = np.asarray(blocks[blk]['events'], np.int64)
        pair_idx = np.stack([rows, rows + 1], axis=1)       # [E, 2]
        xbv = xz[:, pair_idx, :]                            # [C, E, 2, W]
        xbv = np.ascontiguousarray(xbv.transpose(1, 2, 0, 3))  # [E,2,C,W]
        in_maps.append({
            'xb': xbv,
            'lt': lt_blk[blk],
            'lts': lts_on if blk == 0 else lts_off,
            'blkid': np.array([[blk]], np.int32),
            'coefr': coef_on if blk == 0 else coef_off,
            'biasd': biasd,
        })
    return in_maps


def kernel(x, weight, bias):
    from concourse.bass_utils import run_bass_kernel_spmd
    x = np.asarray(x, dtype=np.float32)
    weight = np.asarray(weight, dtype=np.float32)
    bias = np.asarray(bias, dtype=np.float32)

    nc, tt, blocks, E, scs = _get_compiled()
    in_maps = _core_inputs(x, weight, bias, tt, blocks, E, scs)
    res = run_bass_kernel_spmd(nc, in_maps, core_ids=list(range(NCORES)))

    out = np.empty((B, O, H, W), np.float32)
    for cid in range(NCORES):
        b, blk = cid // 4, cid % 4
        oc = res.results[cid]['out']                        # [NROW, O, W]
        out[b, :, blk * NROW:(blk + 1) * NROW, :] = oc.transpose(1, 0, 2)
    return out
